# revision 11
# baseline (speedup 1.0000x reference)
"""ConformerAttention (relative-position MHA, Transformer-XL shift) on 8 trn2
NeuronCores, data-parallel over the batch (B=8 -> 1 batch element per core).

Layout strategy per core (batch element b, S=1024, D=512, H=8, hd=64):
  - LayerNorm token-partitioned, then h^T via PE "transpose" matmuls.
  - q/k projections produce d-partitioned q^T/k^T (weights pre-transposed on
    host); v token-partitioned with an extra all-ones column per head
    (augmented Wv) so the attention-weight row-sum falls out of the AV matmul.
  - Scores are built TRANSPOSED [k, q] so AV needs no big transpose:
      content^T via (k^T stationary, q^T+bias_u moving) matmuls,
      pos scores via banded (q^T+bias_v) @ p^T matmuls -> bf16 -> HBM ->
      re-read with row stride (W-1) which realizes the rel-shift -> merged
      into the content PSUM with regular matmuls against an identity
      (lhsT=pos_block, rhs=I  =>  += pos_block^T).
  - softmax: exp on ScalarE (scale=1/sqrt(hd) folded in); row sums come from
    the augmented ones column of V during the AV matmul; normalization is a
    single fused multiply at the AV-PSUM exit using a partition-broadcast of
    1/sumexp.
  - out-proj with host-transposed Wo + bias via a K=1 ones-row matmul;
    residual added at PSUM exit in fp32.
All matmul operands bf16 (fp32 PSUM accumulation); LN/residual fp32.
"""

import numpy as np
import ml_dtypes

import concourse.bass as bass
import concourse.mybir as mybir
import concourse.tile as tile
from concourse.bass_utils import run_bass_kernel_spmd
from concourse.masks import make_identity
from concourse.vector_clock import ScopedClock

# ---------------------------------------------------------------------------
# Workaround: external neuronxcc walrus rejects >1 sync-wait command on a
# TPB_CTRL instruction; Tile's end-of-context drain collects one wait per live
# logical processor.  Split the waits across preceding NOPs on the sync engine.
_MAX_WAITS = 1


def _drain_and_barrier_split(self, tick_clock, wait_clock):
    nc = self.nc
    collector = nc.sync.nop(nofuse=True)
    wait_clock.add_sem_waits(
        collector.ins, ScopedClock({None: tick_clock.global_clock})
    )
    si = collector.ins.sync_info
    waits = list(si.on_wait) if si is not None and si.on_wait else []
    if len(waits) > _MAX_WAITS:
        si.on_wait = waits[:_MAX_WAITS]
        rest = waits[_MAX_WAITS:]
        while rest:
            extra = nc.sync.nop(nofuse=True)
            chunk, rest = rest[:_MAX_WAITS], rest[_MAX_WAITS:]
            esi = extra.ins.sync_info
            if esi is None:
                extra.ins.sync_info = mybir.SyncInfo(on_wait=chunk, on_update=[])
            else:
                esi.on_wait = chunk
    nc.sync.drain()

    nc.all_engine_barrier()
    assert self.sems is not None
    popped = nc._tile_sem_poison_stack.pop()
    assert popped is self._sem_poison
    nc.clear_and_free_semaphores(list(self.sems.allocated().values()))
    nc.all_engine_barrier()


tile.TileContext._drain_and_barrier = _drain_and_barrier_split


def _split_sync_waits(nc):
    """External neuronxcc walrus accepts at most ONE sync-wait command per
    instruction.  For any instruction carrying k>1 waits, inject k-1 NOPs on
    the same engine immediately before it, each carrying one of the waits.
    Same-processor program order makes this semantically equivalent."""
    needs = []  # (block, inst) with >1 wait
    for bb in nc.main_func.blocks:
        for ins in bb.instructions:
            if ins.sync_info and ins.sync_info.on_wait and len(ins.sync_info.on_wait) > 1:
                needs.append((bb, ins))
    if not needs:
        return
    # create nop carriers (they append to the current bb; we relocate them)
    carriers = {}
    for bb, ins in needs:
        k = len(ins.sync_info.on_wait) - 1
        lst = []
        for _ in range(k):
            n = nc.engines[ins.engine].nop(nofuse=True)
            lst.append(n.ins)
        carriers[ins.name] = lst
    carrier_names = {c.name for lst in carriers.values() for c in lst}
    # remove carriers from wherever they were appended
    for bb in nc.main_func.blocks:
        insts = [i for i in bb.instructions if i.name not in carrier_names]
        if len(insts) != len(bb.instructions):
            bb.instructions = insts
    # insert carriers before their targets, moving waits
    for bb, ins in needs:
        lst = carriers[ins.name]
        waits = list(ins.sync_info.on_wait)
        ins.sync_info.on_wait = waits[:1]
        for j, c in enumerate(lst):
            c.sync_info = mybir.SyncInfo(on_wait=[waits[1 + j]], on_update=[])
        insts = bb.instructions
        idx = next(i for i, x in enumerate(insts) if x.name == ins.name)
        bb.instructions = insts[:idx] + lst + insts[idx:]


# ---------------------------------------------------------------------------

F32 = mybir.dt.float32
BF16 = mybir.dt.bfloat16
BF = ml_dtypes.bfloat16

B, S, D = 8, 1024, 512
H, HD = 8, 64
NT = S // 128          # 8 token tiles
ND = D // 128          # 4 d-model tiles
P = 2 * S - 1          # 2047
P2 = 2048              # padded relative-position length
NB = 1152              # pos band width per q-tile (needs 1151)
VA = 520               # augmented v width: 8 heads * (64 + 1 ones col)
SCALE = 1.0 / np.sqrt(HD)

_CACHE = {}

def _exit_copy(nc, idx, out, in_):
    """Alternate PSUM->SBUF exit copies between ScalarE and VectorE."""
    if idx % 2 == 0:
        nc.scalar.copy(out=out, in_=in_)
    else:
        nc.vector.tensor_copy(out=out, in_=in_)



def _build():
    nc = bass.Bass()

    x_d = nc.declare_dram_parameter("x", [S, D], F32, isOutput=False)
    wqT_d = nc.declare_dram_parameter("wqT", [D, D], BF16, isOutput=False)
    wkT_d = nc.declare_dram_parameter("wkT", [D, D], BF16, isOutput=False)
    wvTa_d = nc.declare_dram_parameter("wvTa", [D, VA], BF16, isOutput=False)
    woT_d = nc.declare_dram_parameter("woT", [D, D], BF16, isOutput=False)
    wpT_d = nc.declare_dram_parameter("wpT", [D, D], BF16, isOutput=False)
    peT_d = nc.declare_dram_parameter("peT", [D, P2], BF16, isOutput=False)
    bqu_d = nc.declare_dram_parameter("bqu", [D], F32, isOutput=False)
    bqv_d = nc.declare_dram_parameter("bqv", [D], F32, isOutput=False)
    bk_d = nc.declare_dram_parameter("bk", [D], F32, isOutput=False)
    bva_d = nc.declare_dram_parameter("bva", [1, VA], BF16, isOutput=False)
    bo_d = nc.declare_dram_parameter("bo", [1, D], BF16, isOutput=False)
    y_d = nc.declare_dram_parameter("y", [S, D], F32, isOutput=True)

    with tile.TileContext(nc) as tc:
        with (
            tc.tile_pool(name="consts", bufs=1) as consts,
            tc.tile_pool(name="persist", bufs=1) as persist,
            tc.tile_pool(name="dram", bufs=1, space="DRAM") as dram,
        ):
            # --- constants -------------------------------------------------
            ident = consts.tile([128, 128], BF16, tag="ident", name="ident")
            make_identity(nc, ident)
            ones_row = consts.tile([1, 128], BF16, tag="ones_row", name="ones_row")
            nc.vector.memset(ones_row, 1.0)
            eps_t = consts.tile([128, 1], F32, tag="eps", name="eps")
            nc.vector.memset(eps_t, 1e-5)
            bva_t = consts.tile([1, VA], BF16, tag="bva", name="bva")
            nc.sync.dma_start(out=bva_t, in_=bva_d[:, :])
            bo_t = consts.tile([1, D], BF16, tag="bo", name="bo")
            nc.sync.dma_start(out=bo_t, in_=bo_d[:, :])
            bqu_t, bqv_t, bk_t = [], [], []
            for m in range(ND):
                for lst, src in ((bqu_t, bqu_d), (bqv_t, bqv_d), (bk_t, bk_d)):
                    t = consts.tile([128, 1], F32, tag=f"bias{len(lst)}_{m}", name=f"bias{len(lst)}_{m}")
                    nc.sync.dma_start(out=t, in_=src[m * 128:(m + 1) * 128].unsqueeze(1))
                    lst.append(t)

            # --- persistent activations -----------------------------------
            x_t = [persist.tile([128, D], F32, tag=f"x{i}", name=f"x{i}") for i in range(NT)]
            hT = [persist.tile([128, S], BF16, tag=f"hT{m}", name=f"hT{m}") for m in range(ND)]
            quT = [persist.tile([128, S], BF16, tag=f"quT{m}", name=f"quT{m}") for m in range(ND)]
            qvT = [persist.tile([128, S], BF16, tag=f"qvT{m}", name=f"qvT{m}") for m in range(ND)]
            kT = [persist.tile([128, S], BF16, tag=f"kT{m}", name=f"kT{m}") for m in range(ND)]
            vA = [persist.tile([128, VA], BF16, tag=f"v{i}", name=f"v{i}") for i in range(NT)]
            pT = [persist.tile([128, P2], BF16, tag=f"pT{m}", name=f"pT{m}") for m in range(ND)]
            outT = [persist.tile([128, S], BF16, tag=f"outT{m}", name=f"outT{m}") for m in range(ND)]

            r_dram = dram.tile([H, NT, 128, NB], BF16, tag="r_scratch", name="r_scratch")

            # --- phase 1: load x, LayerNorm, h^T --------------------------
            with (
                tc.tile_pool(name="ln_sb", bufs=3) as ln_sb,
                tc.tile_pool(name="ln_ps", bufs=3, space="PSUM") as ln_ps,
            ):
                hs_t = []
                for i in range(NT):
                    nc.sync.dma_start(out=x_t[i], in_=x_d[i * 128:(i + 1) * 128, :])
                    stats = ln_sb.tile([128, 6], F32, tag="stats", name="stats")
                    nc.vector.bn_stats(out=stats, in_=x_t[i])
                    mv = ln_sb.tile([128, 2], F32, tag="mv", name="mv")
                    nc.vector.bn_aggr(out=mv, in_=stats)
                    std = ln_sb.tile([128, 1], F32, tag="std", name="std")
                    nc.scalar.activation(
                        out=std, in_=mv[:, 1:2],
                        func=mybir.ActivationFunctionType.Sqrt,
                        bias=eps_t, scale=1.0,
                    )
                    rstd = ln_sb.tile([128, 1], F32, tag="rstd", name="rstd")
                    nc.vector.reciprocal(out=rstd, in_=std)
                    hs = ln_sb.tile([128, D], BF16, tag="hs", name="hs")
                    nc.vector.tensor_scalar(
                        out=hs, in0=x_t[i],
                        scalar1=mv[:, 0:1], scalar2=rstd,
                        op0=mybir.AluOpType.subtract,
                        op1=mybir.AluOpType.mult,
                    )
                    hs_t.append(hs)
                    # h^T blocks: psum[dm, tok] = hs[:, dm].T
                    for m in range(ND):
                        ps = ln_ps.tile([128, 128], F32, tag="htp", name="htp")
                        nc.tensor.matmul(
                            ps, lhsT=hs[:, m * 128:(m + 1) * 128], rhs=ident,
                            start=True, stop=True,
                        )
                        _exit_copy(nc, i + m, hT[m][:, i * 128:(i + 1) * 128], ps)

            # --- phase 2: projections -------------------------------------
            with (
                tc.tile_pool(name="pj_w", bufs=1) as pj_w,
                tc.tile_pool(name="pj_ps", bufs=2, space="PSUM") as pj_ps,
                tc.tile_pool(name="pj_ps_v", bufs=1, space="PSUM") as pj_ps_v,
            ):
                wq_t = [pj_w.tile([128, D], BF16, tag=f"wq{k}", name=f"wq{k}") for k in range(ND)]
                wk_t = [pj_w.tile([128, D], BF16, tag=f"wk{k}", name=f"wk{k}") for k in range(ND)]
                wv_t = [pj_w.tile([128, VA], BF16, tag=f"wv{k}", name=f"wv{k}") for k in range(ND)]
                wp_t = [pj_w.tile([128, D], BF16, tag=f"wp{k}", name=f"wp{k}") for k in range(ND)]
                pe_t = [pj_w.tile([128, P2], BF16, tag=f"pe{k}", name=f"pe{k}") for k in range(ND)]
                for k in range(ND):
                    sl = slice(k * 128, (k + 1) * 128)
                    nc.sync.dma_start(out=wq_t[k], in_=wqT_d[sl, :])
                    nc.sync.dma_start(out=wk_t[k], in_=wkT_d[sl, :])
                    nc.sync.dma_start(out=wv_t[k], in_=wvTa_d[sl, :])
                    nc.sync.dma_start(out=wp_t[k], in_=wpT_d[sl, :])
                    nc.sync.dma_start(out=pe_t[k], in_=peT_d[sl, :])

                # q^T, k^T : [dout(m) partitions, tok]
                for m in range(ND):
                    msl = slice(m * 128, (m + 1) * 128)
                    for c in range(2):
                        csl = slice(c * 512, (c + 1) * 512)
                        psq = pj_ps.tile([128, 512], F32, tag="psq", name="psq")
                        psk = pj_ps.tile([128, 512], F32, tag="psk", name="psk")
                        for k in range(ND):
                            nc.tensor.matmul(
                                psq, lhsT=wq_t[k][:, msl],
                                rhs=hT[k][:, csl], start=(k == 0), stop=(k == ND - 1),
                            )
                        for k in range(ND):
                            nc.tensor.matmul(
                                psk, lhsT=wk_t[k][:, msl],
                                rhs=hT[k][:, csl], start=(k == 0), stop=(k == ND - 1),
                            )
                        nc.vector.tensor_scalar_add(
                            out=quT[m][:, csl], in0=psq, scalar1=bqu_t[m])
                        nc.vector.tensor_scalar_add(
                            out=qvT[m][:, csl], in0=psq, scalar1=bqv_t[m])
                        nc.vector.tensor_scalar_add(
                            out=kT[m][:, csl], in0=psk, scalar1=bk_t[m])

                # v (token-partitioned, augmented)
                for i in range(NT):
                    isl = slice(i * 128, (i + 1) * 128)
                    psv = pj_ps_v.tile([128, VA], F32, tag="psv", name="psv")
                    for c, (off, sz) in enumerate(((0, 512), (512, VA - 512))):
                        vsl = slice(off, off + sz)
                        for k in range(ND):
                            nc.tensor.matmul(
                                psv[:, vsl], lhsT=hT[k][:, isl],
                                rhs=wv_t[k][:, vsl], start=(k == 0), stop=False,
                            )
                        nc.tensor.matmul(
                            psv[:, vsl], lhsT=ones_row, rhs=bva_t[:, vsl],
                            start=False, stop=True,
                        )
                    _exit_copy(nc, i, vA[i], psv)

                # p^T : [dout(m) partitions, P2]
                for m in range(ND):
                    msl = slice(m * 128, (m + 1) * 128)
                    for c in range(P2 // 512):
                        csl = slice(c * 512, (c + 1) * 512)
                        psp = pj_ps.tile([128, 512], F32, tag="psp", name="psp")
                        for k in range(ND):
                            nc.tensor.matmul(
                                psp, lhsT=wp_t[k][:, msl], rhs=pe_t[k][:, csl],
                                start=(k == 0), stop=(k == ND - 1),
                            )
                        _exit_copy(nc, m + c, pT[m][:, csl], psp)

            # --- phase 3: pos score bands -> DRAM (rel-shift roundtrip) ---
            with (
                tc.tile_pool(name="r_sb", bufs=4) as r_sb,
                tc.tile_pool(name="r_ps", bufs=4, space="PSUM") as r_ps,
            ):
                for h in range(H):
                    hp, hh = h // 2, h % 2
                    rows = slice(64 * hh, 64 * hh + 64)
                    for a in range(NT):
                        p0 = S - 1 - 128 * a - 127
                        rsb = r_sb.tile([128, NB], BF16, tag="rsb", name="rsb")
                        for off, sz in ((0, 512), (512, 512), (1024, NB - 1024)):
                            psr = r_ps.tile([128, 512], F32, tag="psr", name="psr")
                            nc.tensor.matmul(
                                psr[:, :sz],
                                lhsT=qvT[hp][rows, a * 128:(a + 1) * 128],
                                rhs=pT[hp][rows, p0 + off: p0 + off + sz],
                                start=True, stop=True,
                                tile_position=(64 * hh, 0),
                            )
                            _exit_copy(nc, off // 512 + a, rsb[:, off:off + sz], psr[:, :sz])
                        nc.sync.dma_start(out=r_dram[h, a], in_=rsb)

            # --- phase 4: scores^T, softmax, AV ---------------------------
            with (
                tc.tile_pool(name="pos_sh", bufs=10) as pos_pool,
                tc.tile_pool(name="attn", bufs=10) as attn_pool,
                tc.tile_pool(name="sm_sb", bufs=4) as sm_sb,
                tc.tile_pool(name="sc_ps", bufs=2, space="PSUM") as sc_ps,
                tc.tile_pool(name="av_ps", bufs=2, space="PSUM") as av_ps,
            ):
                for h in range(H):
                    hp, hh = h // 2, h % 2
                    rows = slice(64 * hh, 64 * hh + 64)
                    # shifted readback: pos_sh[a][q, k] = band[q, 127 - q + k]
                    pos_sh = []
                    for a in range(NT):
                        psh = pos_pool.tile([128, S], BF16, tag="psh", name="psh")
                        base = ((h * NT + a) * 128) * NB + 127
                        src = bass.AP(
                            tensor=r_dram.tensor, offset=r_dram.offset + base,
                            ap=[[NB - 1, 128], [1, S]],
                        )
                        nc.sync.dma_start(out=psh, in_=src)
                        pos_sh.append(psh)

                    attnT = []
                    for t in range(NT):
                        ps = sc_ps.tile([128, S], F32, tag="sc", name="sc")
                        tsl = slice(t * 128, (t + 1) * 128)
                        for c in range(2):
                            csl = slice(c * 512, (c + 1) * 512)
                            nc.tensor.matmul(
                                ps[:, csl], lhsT=kT[hp][rows, tsl],
                                rhs=quT[hp][rows, csl], start=True, stop=False,
                                tile_position=(64 * hh, 0),
                            )
                        for a in range(NT):
                            nc.tensor.matmul(
                                ps[:, a * 128:(a + 1) * 128],
                                lhsT=pos_sh[a][:, tsl], rhs=ident,
                                start=False, stop=(True),
                            )
                        at = attn_pool.tile([128, S], BF16, tag="at", name="at")
                        nc.scalar.activation(
                            out=at, in_=ps,
                            func=mybir.ActivationFunctionType.Exp,
                            bias=0.0, scale=float(SCALE),
                        )
                        attnT.append(at)

                    av = av_ps.tile([65, S], F32, tag="av", name="av")
                    vcols = slice(65 * h, 65 * h + 65)
                    for t in range(NT):
                        for c in range(2):
                            csl = slice(c * 512, (c + 1) * 512)
                            nc.tensor.matmul(
                                av[:, csl], lhsT=vA[t][:, vcols],
                                rhs=attnT[t][:, csl],
                                start=(t == 0), stop=(t == NT - 1),
                            )
                    recip = sm_sb.tile([1, S], F32, tag="recip", name="recip")
                    nc.vector.reciprocal(out=recip, in_=av[64:65, :])
                    rdr = dram.tile([1, S], F32, tag=f"recip_dram{h}",
                                    name=f"recip_dram{h}")
                    nc.sync.dma_start(out=rdr, in_=recip)
                    rb = sm_sb.tile([64, S], F32, tag="rb", name="rb")
                    rb_src = bass.AP(
                        tensor=rdr.tensor, offset=rdr.offset,
                        ap=[[0, 64], [1, S]],
                    )
                    nc.sync.dma_start(out=rb, in_=rb_src)
                    nc.vector.tensor_tensor(
                        out=outT[hp][rows, :], in0=av[0:64, :], in1=rb,
                        op=mybir.AluOpType.mult,
                    )

            # --- phase 5: out-proj + residual -----------------------------
            with (
                tc.tile_pool(name="op_w", bufs=1) as op_w,
                tc.tile_pool(name="op_sb", bufs=3) as op_sb,
                tc.tile_pool(name="op_ps", bufs=3, space="PSUM") as op_ps,
            ):
                wo_t = [op_w.tile([128, D], BF16, tag=f"wo{k}", name=f"wo{k}") for k in range(ND)]
                for k in range(ND):
                    nc.sync.dma_start(out=wo_t[k], in_=woT_d[k * 128:(k + 1) * 128, :])
                for i in range(NT):
                    isl = slice(i * 128, (i + 1) * 128)
                    pso = op_ps.tile([128, D], F32, tag="pso", name="pso")
                    for k in range(ND):
                        nc.tensor.matmul(
                            pso, lhsT=outT[k][:, isl], rhs=wo_t[k],
                            start=(k == 0), stop=False,
                        )
                    nc.tensor.matmul(
                        pso, lhsT=ones_row, rhs=bo_t, start=False, stop=True,
                    )
                    y_sb = op_sb.tile([128, D], F32, tag="ysb", name="ysb")
                    nc.vector.tensor_tensor(
                        out=y_sb, in0=pso, in1=x_t[i], op=mybir.AluOpType.add,
                    )
                    nc.sync.dma_start(out=y_d[isl, :], in_=y_sb)

    _split_sync_waits(nc)
    return nc


def _host_prep(inputs):
    ln_w = np.asarray(inputs["ln_w"], np.float32)
    ln_b = np.asarray(inputs["ln_b"], np.float32)
    Wq = np.asarray(inputs["Wq"], np.float32)
    Wk = np.asarray(inputs["Wk"], np.float32)
    Wv = np.asarray(inputs["Wv"], np.float32)
    Wo = np.asarray(inputs["Wo"], np.float32)
    Wp = np.asarray(inputs["Wp"], np.float32)

    # fold LN affine into the consumer projections: h' = hs * w + b
    wqT = (Wq * ln_w).T.copy()          # [din, dout]
    wkT = (Wk * ln_w).T.copy()
    wvT = (Wv * ln_w).T.copy()
    bq_e = np.asarray(inputs["bq"], np.float32) + Wq @ ln_b
    bk_e = np.asarray(inputs["bk"], np.float32) + Wk @ ln_b
    bv_e = np.asarray(inputs["bv"], np.float32) + Wv @ ln_b

    # augmented Wv^T: per head 64 cols + a zeros col whose bias row is 1.0
    wvTa = np.zeros((D, VA), np.float32)
    bva = np.zeros((VA,), np.float32)
    for h in range(H):
        wvTa[:, 65 * h:65 * h + 64] = wvT[:, 64 * h:64 * h + 64]
        bva[65 * h:65 * h + 64] = bv_e[64 * h:64 * h + 64]
        bva[65 * h + 64] = 1.0

    peT = np.zeros((D, P2), np.float32)
    peT[:, :P] = np.asarray(inputs["pos_emb"], np.float32).T

    return {
        "wqT": wqT.astype(BF), "wkT": wkT.astype(BF),
        "wvTa": wvTa.astype(BF), "woT": Wo.T.copy().astype(BF),
        "wpT": Wp.T.copy().astype(BF), "peT": peT.astype(BF),
        "bqu": bq_e + np.asarray(inputs["pos_bias_u"], np.float32),
        "bqv": bq_e + np.asarray(inputs["pos_bias_v"], np.float32),
        "bk": bk_e,
        "bva": bva.reshape(1, VA).astype(BF),
        "bo": np.asarray(inputs["bo"], np.float32).reshape(1, D).astype(BF),
    }


def kernel(**inputs) -> np.ndarray:
    import os

    if "nc" not in _CACHE:
        _CACHE["nc"] = _build()
    nc = _CACHE["nc"]

    shared = _host_prep(inputs)
    x = np.ascontiguousarray(np.asarray(inputs["x"], np.float32))
    in_maps = [dict(shared, x=x[b]) for b in range(B)]
    kw = {}
    if os.environ.get("KERNEL_TRACE"):
        kw = dict(trace=True, tmpdir=os.environ.get("KERNEL_TRACE_DIR") or None)
    res = run_bass_kernel_spmd(nc, in_maps, core_ids=list(range(B)), **kw)
    _CACHE["last_res"] = res
    return np.stack([res.results[b]["y"] for b in range(B)], axis=0)
